# revision 73
# baseline (speedup 1.0000x reference)
"""Trainium2 Bass kernel for nn_Attention_17532056502607.

Multi-head self-attention (B=8, N=48*48=2304 tokens, C=384, 8 heads of 48):
    q = x @ q_w.T + q_b ; k,v = x @ kv_w.T + kv_b
    out = softmax(q k^T / sqrt(48)) v ; y = out @ proj_w.T + proj_b

Sharding: data-parallel, one batch element per NeuronCore (8 cores).

The kernel is scheduled around the Activation engine: the softmax exp over
8 heads x 2304^2 scores (331,776 free-dim elements at ~0.83 ns each) is the
hard floor, so every other engine's work is interleaved UNDER the exp stream:

  - lead-in is only k(p0, all token chunks) + q(p0, chunk0); the first S^T
    group issues ~15 us in.  v tiles, the remaining q/k projections, and the
    output projection are emitted as "aux units" at score-group boundaries
    inside the attention loop, filling PE slack while ACT stays saturated.
  - per (head-pair, query-chunk): S^T = kT.T @ qT in groups of 3/2 PSUM
    banks (double-buffered), exp on ACT (PSUM->SBUF bf16), attn@V lagged 3
    groups so the PE never waits on exp latency.
  - attn@V accumulates both heads of a pair into ONE PSUM bank (partitions
    0:49 / 64:113 - disjoint column tiles keep the two matmuls concurrent),
    with each head's V block [ones | v] so row 0/64 carries the softmax
    denominator.
  - normalize is prompt (same chunk): reciprocal_approx_fast of the two
    denominator rows, one K=2 selector matmul broadcasts both recips into a
    PSUM tile, two DVE muls write the normalized outT. No oU staging copy.
  - output projection y = sum_pairs outT-pair.T @ projw-pair + bias runs as
    aux units one chunk behind pair 3, K=113 spanning both head blocks with
    zero weight rows under the denominators.

Matmul dtypes: float32r for x->q/k/v and the output projection, bf16 for the
attention core (rel err ~2.7e-3 vs the fp32 reference).
"""

import os
import sys
from collections import deque

import numpy as np

for _p in ("/opt/trn_rl_repo",):
    if _p not in sys.path:
        sys.path.append(_p)

import concourse.bass as bass  # noqa: E402
import concourse.tile as tile  # noqa: E402
from concourse import bacc, mybir  # noqa: E402
from concourse.bass_utils import run_bass_kernel_spmd  # noqa: E402

# ---------------------------------------------------------------- DVE exp
# A softmax-exp for the Vector engine so part of the exp stream runs off the
# (otherwise saturating) Activation engine:  exp(s) = (cubic(s*SCALE/32))^32.
# The SCALE/32 factor is folded into the K projection weights host-side, so
# scores arrive pre-scaled; op1 is a Horner cubic of e^x on [-0.23, 0.23]
# (max rel err 2.4e-5), op2 raises it to the 32nd power by five squarings
# (overall max rel err ~6e-4 for |s|<=7.3 -- far inside the bf16 budget).
from concourse import dve_ops as _D  # noqa: E402
from concourse.dve_spec import (  # noqa: E402
    C0 as _C0,
    C1 as _C1,
    C2 as _C2,
    C3 as _C3,
    Spec as _Spec,
    Src0 as _Src0,
    _has_src1,
    _spill_c3_to_src1,
    lower as _dve_lower,
    sq as _sq,
)
from concourse.dve_uop import DveOpSpec as _DveOpSpec  # noqa: E402

EXP_A = (0.99998967, 1.00003049, 0.50193648, 0.16607283)  # a0..a3


def _register_dve_op(name, spec):
    existing = {op.name: op for op in _D.OPS}
    if name in existing:
        return existing[name]
    row = max(_D._SUB_OPCODE_FOR_NAME.values()) + 1
    _D._SUB_OPCODE_FOR_NAME[name] = row
    shas = {}
    for ver in ("v3", "v4"):
        s = _DveOpSpec(
            name=name, opcode=row, uops=_dve_lower(spec, ver=ver),
            rd1_en=_has_src1(spec),
        )
        shas[ver] = s.sha(ver)
    op = _D.DveOp(name, spec, subdim=False, uops_sha=shas)
    _D.OPS.append(op)
    return op


# p = ((a3*x + a2)*x + a1)*x + a0   (C0=a3, C1=a2, C2=a1, C3=a0 via in1)
EXP_POLY = _register_dve_op(
    "ANT_EXP_POLY",
    _Spec(body=_spill_c3_to_src1(((_Src0 * _C0 + _C1) * _Src0 + _C2) * _Src0 + _C3)),
)
# r = p^32 by five squarings
EXP_POW32 = _register_dve_op(
    "ANT_EXP_POW32", _Spec(body=_sq(_sq(_sq(_sq(_sq(_Src0))))))
)

# ---------------------------------------------------------------- constants
B = 8
HH = 48
WW = 48
C = 384
N = HH * WW  # 2304
NH = 8
HD = 48
PAIRS = NH // 2  # 4
P = 128
NT = N // P  # 18 token tiles
KTC = C // P  # 3 contraction tiles over C
SCALE = float(HD) ** -0.5
VW = NH * (HD + 1)  # 392: v with a ones column per head
XW = N + 512 + 512 + VW  # 3720: packed [x^T | wk | wq | wv] row width
CHUNKS = [(0, 512), (512, 512), (1024, 512), (1536, 512), (2048, 256)]

F32 = mybir.dt.float32
MM_DT = getattr(mybir.dt, os.environ.get("ATTN_MM_DT", "bfloat16"))
AV_DT = (
    mybir.dt.bfloat16
    if MM_DT == mybir.dt.float32r
    else getattr(mybir.dt, os.environ.get("ATTN_AV_DT", MM_DT.value))
)
ST_DT = getattr(
    mybir.dt,
    os.environ.get(
        "ATTN_ST_DT",
        "bfloat16" if MM_DT == mybir.dt.float32r else MM_DT.value,
    ),
)
BC_DT = mybir.dt.float32r if MM_DT != mybir.dt.float32 else F32

_EXP = mybir.ActivationFunctionType.Exp


def _emit(tc: tile.TileContext, d: dict, ctx):
    nc = tc.nc

    persist = ctx.enter_context(tc.tile_pool(name="persist", bufs=1))
    v_sb = persist.tile([P, NT, VW], AV_DT, name="v_sb")
    qT_sb = persist.tile([P, PAIRS, N], ST_DT, name="qT_sb")
    kT_sb = persist.tile([P, PAIRS, N], ST_DT, name="kT_sb")
    oT_sb = persist.tile([P, PAIRS, N], MM_DT, name="oT_sb")
    y_sb = persist.tile([P, NT, C], F32, name="y_sb")
    pw_sb = persist.tile([P, PAIRS, C], MM_DT, name="pw_sb")
    qkb_sb = persist.tile([P, 2 * PAIRS], F32, name="qkb_sb")
    qb_sb = qkb_sb[:, 0:PAIRS]
    kb_sb = qkb_sb[:, PAIRS : 2 * PAIRS]
    vb_sb = persist.tile([1, VW], MM_DT, name="vb_sb")
    pb_sb = persist.tile([1, C], MM_DT, name="pb_sb")
    selE_sb = persist.tile([1, 256], BC_DT, name="selE_sb")
    ones32 = persist.tile([1, P], F32, name="ones32")
    a0_sb = persist.tile([P, 1], F32, name="a0_sb")
    # phase-A operands persist through the attention loop: the q/k/v
    # projections for pairs 1-3 are emitted as aux units inside it.
    # x^T, wk, wq, wv are packed into ONE host tensor so the lead-in needs
    # only three DMA triggers (the Sync queue serializes triggers at ~700ns
    # each), split per contraction tile so k(p0) starts on the first third.
    xkqv_sb = persist.tile([P, KTC, XW], MM_DT, name="xkqv_sb")

    for kt in range(KTC):
        nc.sync.dma_start(
            xkqv_sb[:, kt, :], d["xT"][kt * P : (kt + 1) * P, :]
        )
    nc.sync.dma_start(pw_sb[:], d["pwP"].rearrange("r p m -> p r m"))
    nc.sync.dma_start(qkb_sb[:], d["qkbP"])
    nc.sync.dma_start(vb_sb[:], d["vbA"])
    nc.sync.dma_start(pb_sb[:], d["pbR"])
    nc.sync.dma_start(selE_sb[:], d["selE"])
    # one-time memsets run on the (otherwise idle) GPSIMD engine so the DVE
    # queue stays clear for the lead-in projection drains
    nc.gpsimd.memset(ones32[:], 1.0)
    nc.gpsimd.memset(a0_sb[:], EXP_A[0])
    # dummy exp primes the ACT table load (~2.7us DMA) during the lead-in,
    # off the critical path of the first real score group; self-contained on
    # the ACT engine (memzero then exp) so it issues immediately
    dummy_sb = persist.tile([1, 1], F32, name="dummy_sb")
    nc.scalar.memzero(dummy_sb[0:1, 0:1])
    nc.scalar.activation(dummy_sb[0:1, 0:1], dummy_sb[0:1, 0:1], _EXP)
    if MM_DT == mybir.dt.bfloat16:
        ones_mm = persist.tile([1, P], MM_DT, name="ones_mm")
        nc.gpsimd.memset(ones_mm[:], 1.0)
    elif MM_DT == mybir.dt.float32r:
        ones_mm = ones32.bitcast(MM_DT)
    else:
        ones_mm = ones32

    def _proj(pool, pr, q0, qw, which):
        ps = pool.tile([P, 512], F32, name="psqk", tag="aux")
        w_off = N + (512 if which == "q" else 0)
        b_sb = qb_sb if which == "q" else kb_sb
        dst = qT_sb if which == "q" else kT_sb
        for kt in range(KTC):
            nc.tensor.matmul(
                ps[:, 0:qw],
                lhsT=xkqv_sb[:, kt, w_off + pr * P : w_off + (pr + 1) * P],
                rhs=xkqv_sb[:, kt, q0 : q0 + qw],
                start=(kt == 0),
                stop=(kt == KTC - 1),
            )
        nc.vector.tensor_scalar_add(
            dst[:, pr, q0 : q0 + qw], ps[:, 0:qw], b_sb[:, pr : pr + 1]
        )

    # ---------------- lead-in: k(p0) full, q(p0, c0) --------------------
    # a dedicated double-buffered PSUM pool (closed before the attention
    # pools open) lets the lead-in projections pipeline their drains
    with tc.tile_pool(name="lead", bufs=2, space="PSUM") as lead_ps:
        for q0, qw in CHUNKS:
            _proj(lead_ps, 0, q0, qw, "k")
        _proj(lead_ps, 0, CHUNKS[0][0], CHUNKS[0][1], "q")

    with (
        tc.tile_pool(name="es", bufs=7) as es_pool,
        tc.tile_pool(name="rcp", bufs=2) as rc_pool,
        tc.tile_pool(name="psS", bufs=1, space="PSUM") as psS,
        tc.tile_pool(name="psO", bufs=2, space="PSUM") as psO,
        tc.tile_pool(name="aux", bufs=2, space="PSUM") as aux_ps,
        tc.tile_pool(name="fin", bufs=3) as fin_pool,
    ):
        # ---------------- aux units: small PE jobs run at group boundaries
        def qk_unit(pr, q0, qw, which):
            return lambda: _proj(aux_ps, pr, q0, qw, which)

        def v_unit(nt):
            def emit():
                psv = aux_ps.tile([P, 512], F32, name="psv", tag="aux")
                for kt in range(KTC):
                    nc.tensor.matmul(
                        psv[:, 0:VW],
                        lhsT=xkqv_sb[:, kt, nt * P : (nt + 1) * P],
                        rhs=xkqv_sb[:, kt, N + 1024 : XW],
                        start=(kt == 0),
                        stop=False,
                    )
                nc.tensor.matmul(
                    psv[:, 0:VW],
                    lhsT=ones_mm[:, 0:P],
                    rhs=vb_sb[:],
                    start=False,
                    stop=True,
                )
                nc.vector.tensor_copy(v_sb[:, nt, :], psv[:, 0:VW])

            return emit

        def finp_unit(pr, nt):
            # one pair's contribution to the output projection of token tile
            # nt — a single ~0.35us matmul, accumulated across pairs in SBUF
            # (y_sb) by the DVE, so the projection cost spreads evenly over
            # the whole attention loop instead of humping at pair 3.
            def emit():
                fF = aux_ps.tile([P, 512], F32, name="fF", tag="aux")
                nc.tensor.matmul(
                    fF[:, 0:C],
                    lhsT=oT_sb[0:113, pr, nt * P : (nt + 1) * P],
                    rhs=pw_sb[0:113, pr, :],
                    start=True,
                    stop=(pr != 0),
                )
                if pr == 0:
                    nc.tensor.matmul(
                        fF[:, 0:C],
                        lhsT=ones_mm[:, 0:P],
                        rhs=pb_sb[:],
                        start=False,
                        stop=True,
                    )
                    nc.vector.tensor_copy(y_sb[:, nt, :], fF[:, 0:C])
                elif pr < PAIRS - 1:
                    nc.vector.tensor_add(
                        y_sb[:, nt, :], fF[:, 0:C], y_sb[:, nt, :]
                    )
                else:
                    ft = fin_pool.tile([P, C], F32, name="ft", tag="ft")
                    nc.vector.tensor_add(ft[:], fF[:, 0:C], y_sb[:, nt, :])
                    nc.sync.dma_start(d["out"][nt * P : (nt + 1) * P, :], ft[:])

            return emit

        aux_q = deque()

        # big one-time memsets run on GPSIMD, off every critical queue.
        # rows 49-63 / 113-127 of oT_sb are never written by the normalize
        # muls but ARE read (as zeros) by the K=113 output projection.
        _oT_z = oT_sb[:] if MM_DT != mybir.dt.float32r else oT_sb[:].bitcast(F32)
        nc.gpsimd.memset(_oT_z, 0.0)
        # den tiles seed the batched reciprocal; rows 1-63 stay at 1.0 so a
        # single [0:65] reciprocal_approx never sees junk bits
        den_tiles = [
            rc_pool.tile([P, 512], F32, name=f"den{i}", tag=f"den{i}")
            for i in range(2)
        ]
        for dt_ in den_tiles:
            nc.gpsimd.memset(dt_[:], 1.0)
        ci = 0

        # everything else is queued: v tiles first (consumed by attn@V of
        # (p0, c0)), then q(p0, c1..4) get pushed to the FRONT at each chunk
        # start so S^T never waits.
        for nt in range(NT):
            aux_q.append(v_unit(nt))

        # ---------------- attention: flash over q chunks, S^T layout -----
        # 18 groups of 2 (one kt tile, both heads) with two strictly
        # alternating score tags: 4 score banks total, no same-tag chokes
        # anywhere (18 is even, so chunk boundaries alternate too), and a
        # bank left over to double-buffer the aux pool.
        GSIZES = [2] * 18
        # Exp on the Vector engine (ANT_EXP_POLY/POW32) was measured to be a
        # net LOSS: SBUF/PSUM port contention slowed the remaining ACT
        # activations ~19% and the PE matmuls too. Keep the machinery but
        # route nothing.
        DVE_GIS = frozenset()
        pending = None
        av_q = []
        for pr in range(PAIRS):
            for c_i, (q0, qw) in enumerate(CHUNKS):
                # queue next projections front/back as needed
                if c_i + 1 < len(CHUNKS):
                    aux_q.appendleft(qk_unit(pr, *CHUNKS[c_i + 1], "q"))
                if c_i == 0 and pr + 1 < PAIRS:
                    for q0n, qwn in CHUNKS:
                        aux_q.append(qk_unit(pr + 1, q0n, qwn, "k"))
                    aux_q.append(qk_unit(pr + 1, CHUNKS[0][0], CHUNKS[0][1], "q"))

                oT = psO.tile([P, 512], F32, name="oT", tag="oT")
                seq = [(kt, hoff) for kt in range(NT) for hoff in (0, 64)]

                def attnv(est, si, gs, oT=oT, pr=pr, qw=qw):
                    for j in range(gs):
                        kt2, hoff2 = seq[si + j]
                        h = pr * 2 + (0 if hoff2 == 0 else 1)
                        nc.tensor.matmul(
                            oT[hoff2 : hoff2 + HD + 1, 0:qw],
                            lhsT=v_sb[:, kt2, h * (HD + 1) : (h + 1) * (HD + 1)],
                            rhs=est[:, j, 0:qw],
                            start=(kt2 == 0),
                            stop=(kt2 == NT - 1),
                        )

                si = 0
                for gi, gs in enumerate(GSIZES):
                    # attn@V lags 4+ groups (ACT still computing exp of the
                    # recent ones) and the queue carries ACROSS chunk and
                    # pair boundaries: the PE never drains waiting for the
                    # last exps of a chunk, so its pipeline (and pstate)
                    # stays hot. Each closure captured its own oT/pr/qw.
                    maxlag = (
                        2 if (pr == PAIRS - 1 and c_i == len(CHUNKS) - 1) else 4
                    )
                    while len(av_q) > maxlag:
                        av_q.pop(0)()
                    if gi == 6 and pending is not None:
                        # previous chunk's normalize: by gi 6 its last
                        # carried attn@V group (popped at gi <= 4) has
                        # committed the accumulator
                        pending()
                        pending = None
                    tagg = "A" if gi % 2 == 0 else "B"
                    sg = psS.tile([P, gs, 512], F32, name="sg", tag=f"sg{tagg}")
                    for j in range(gs):
                        kt, hoff = seq[si + j]
                        nc.tensor.matmul(
                            sg[:, j, 0:qw],
                            lhsT=kT_sb[hoff : hoff + HD, pr, kt * P : (kt + 1) * P],
                            rhs=qT_sb[hoff : hoff + HD, pr, q0 : q0 + qw],
                            start=True,
                            stop=True,
                        )
                    est = es_pool.tile(
                        [P, gs, 512], AV_DT, name="est", tag="est"
                    )
                    if gi in DVE_GIS:
                        # exp on the Vector engine: cubic then ^32
                        p1t = rc_pool.tile([P, 2, 512], F32, name="p1t", tag="p1")
                        nc.vector._custom_dve(
                            EXP_POLY,
                            out=p1t[:, 0:gs, 0:qw],
                            in0=sg[:, :, 0:qw],
                            in1=a0_sb[:, 0:1],
                            s0=EXP_A[3],
                            s1=EXP_A[2],
                            imm2=EXP_A[1],
                        )
                        nc.vector._custom_dve(
                            EXP_POW32,
                            out=est[:, :, 0:qw],
                            in0=p1t[:, 0:gs, 0:qw],
                        )
                    else:
                        # scores arrive pre-scaled by SCALE/32 (folded into
                        # the K weights host-side)
                        nc.scalar.activation(
                            est[:, :, 0:qw], sg[:, :, 0:qw], _EXP, scale=32.0
                        )
                    av_q.append(
                        lambda est=est, si=si, gs=gs, f=attnv: f(est, si, gs)
                    )
                    # aux units at group boundaries keep the projections and
                    # output flowing under the ACT-bound stream: every other
                    # boundary normally (denser during (p0, c0) to finish
                    # the v tiles in time), plus catch-up when backlogged.
                    if pr == 0 and c_i == 0:
                        # ~one unit per boundary: the v tiles keep pace with
                        # attn@V (3 groups of slack) without starving the
                        # exp stream; 20 slots cover q(c1) + v0..v17 so every
                        # v tile is emitted before the attn@V that reads it
                        n_aux = 2 if gi < 2 else 1
                    elif pr == PAIRS - 1 and c_i == len(CHUNKS) - 1:
                        # drain hard before the tail
                        n_aux = 2
                    else:
                        n_aux = gi % 2 + (1 if len(aux_q) > 10 else 0)
                    for _ in range(n_aux):
                        if aux_q:
                            aux_q.popleft()()
                    si += gs

                # ---------------- deferred normalize ---------------------
                def normalize(oT=oT, pr=pr, q0=q0, qw=qw, ci=ci):
                    # one approx reciprocal covers both denominator rows
                    # (0/64); den rows 1-63 hold 1.0, never junk
                    den = den_tiles[ci % 2]
                    nc.vector.tensor_copy(den[0:1, 0:qw], oT[0:1, 0:qw])
                    nc.vector.tensor_copy(den[64:65, 0:qw], oT[64:65, 0:qw])
                    rec = rc_pool.tile([P, 512], F32, name="rec", tag="rc")
                    nc.vector.reciprocal_approx_fast(
                        rec[0:65, 0:qw], den[0:65, 0:qw]
                    )
                    # both recips cast-copied (fp32->fp32r rounding for the
                    # PE) onto partition 0 of one tile so the selector
                    # matmuls keep base partition 0
                    recW = rc_pool.tile([1, 1024], BC_DT, name="recW", tag="rcW")
                    with nc.allow_low_precision(
                        reason="float32r rounding for the broadcast matmul rhs"
                    ):
                        nc.vector.tensor_copy(recW[0:1, 0:qw], rec[0:1, 0:qw])
                        nc.vector.tensor_copy(
                            recW[0:1, 512 : 512 + qw], rec[64:65, 0:qw]
                        )
                    bcp = aux_ps.tile([P, 512], F32, name="bcp", tag="aux")
                    nc.tensor.matmul(
                        bcp[0:113, 0:qw],
                        lhsT=selE_sb[0:1, 0:113],
                        rhs=recW[0:1, 0:qw],
                        start=True,
                        stop=False,
                    )
                    nc.tensor.matmul(
                        bcp[0:113, 0:qw],
                        lhsT=selE_sb[0:1, 128:241],
                        rhs=recW[0:1, 512 : 512 + qw],
                        start=False,
                        stop=True,
                    )
                    # TensorTensor allows only one PSUM input: stage the
                    # broadcast through SBUF, then multiply oT straight out
                    # of PSUM (no oU staging of the 98 output rows needed)
                    bcs = rc_pool.tile([P, 512], F32, name="bcs", tag="bcs")
                    nc.vector.tensor_copy(bcs[0:113, 0:qw], bcp[0:113, 0:qw])
                    nc.vector.tensor_mul(
                        oT_sb[0 : HD + 1, pr, q0 : q0 + qw],
                        oT[0 : HD + 1, 0:qw],
                        bcs[0 : HD + 1, 0:qw],
                    )
                    nc.vector.tensor_mul(
                        oT_sb[64 : 64 + HD + 1, pr, q0 : q0 + qw],
                        oT[64 : 64 + HD + 1, 0:qw],
                        bcs[64 : 64 + HD + 1, 0:qw],
                    )
                    # this pair's chunk is now final: queue its output-
                    # projection partials (consumed at later boundaries;
                    # FIFO order guarantees pair order per token tile).
                    for nt in range(q0 // P, (q0 + qw) // P):
                        aux_q.append(finp_unit(pr, nt))

                pending = normalize
                ci += 1

        # tail: carried attn@V groups, last chunk's normalize, leftover aux
        for av in av_q:
            av()
        if pending is not None:
            pending()
        while aux_q:
            aux_q.popleft()()


def build_program(n_cores: int = 8):
    nc = bacc.Bacc(
        "TRN2",
        target_bir_lowering=False,
        debug=False,
        enable_asserts=False,
        num_devices=n_cores,
    )
    d = {
        "xT": nc.dram_tensor("xT", [C, XW], MM_DT, kind="ExternalInput").ap(),
        "vbA": nc.dram_tensor("vbA", [1, VW], MM_DT, kind="ExternalInput").ap(),
        "qkbP": nc.dram_tensor("qkbP", [P, 2 * PAIRS], F32, kind="ExternalInput").ap(),
        "pwP": nc.dram_tensor("pwP", [PAIRS, P, C], MM_DT, kind="ExternalInput").ap(),
        "pbR": nc.dram_tensor("pbR", [1, C], MM_DT, kind="ExternalInput").ap(),
        "selE": nc.dram_tensor("selE", [1, 256], BC_DT, kind="ExternalInput").ap(),
        "out": nc.dram_tensor("out", [N, C], F32, kind="ExternalOutput").ap(),
    }
    import contextlib

    with tile.TileContext(nc) as tc:
        with contextlib.ExitStack() as ctx:
            _emit(tc, d, ctx)
    nc.finalize()
    return nc


def _mm_np_dtype():
    if MM_DT == mybir.dt.bfloat16:
        import ml_dtypes

        return ml_dtypes.bfloat16
    return np.float32


def _prep_host(x, q_w, q_b, kv_w, kv_b, proj_w, proj_b):
    """Transpose/pack on host. Returns (per-core xT list, shared map)."""
    f32 = np.float32
    x = np.asarray(x, f32)
    xT = np.ascontiguousarray(x.reshape(B, N, C).transpose(0, 2, 1))  # [B, C, N]

    qwT = np.ascontiguousarray(np.asarray(q_w, f32).T)  # [Cin, Cout]
    kwT = np.ascontiguousarray(np.asarray(kv_w[:C], f32).T)
    vwT = np.ascontiguousarray(np.asarray(kv_w[C:], f32).T)
    pwT = np.ascontiguousarray(np.asarray(proj_w, f32).T)

    # SCALE/32 folds into the K side: scores arrive as s*SCALE/32, matching
    # both the DVE exp (cubic^32) and the ACT path (activation scale=32).
    KS = SCALE / 32.0
    wqP = np.zeros((C, PAIRS * P), f32)
    wkP = np.zeros((C, PAIRS * P), f32)
    qbP = np.zeros((P, PAIRS), f32)
    kbP = np.zeros((P, PAIRS), f32)
    pwP = np.zeros((PAIRS, P, C), f32)
    for p in range(PAIRS):
        a, b = 2 * p, 2 * p + 1
        wqP[:, p * P : p * P + HD] = qwT[:, a * HD : (a + 1) * HD]
        wqP[:, p * P + 64 : p * P + 64 + HD] = qwT[:, b * HD : (b + 1) * HD]
        wkP[:, p * P : p * P + HD] = kwT[:, a * HD : (a + 1) * HD] * KS
        wkP[:, p * P + 64 : p * P + 64 + HD] = kwT[:, b * HD : (b + 1) * HD] * KS
        qbP[0:HD, p] = q_b[a * HD : (a + 1) * HD]
        qbP[64 : 64 + HD, p] = q_b[b * HD : (b + 1) * HD]
        kbP[0:HD, p] = kv_b[a * HD : (a + 1) * HD] * KS
        kbP[64 : 64 + HD, p] = kv_b[b * HD : (b + 1) * HD] * KS
        # rows 1..48 / 65..112 carry the proj weights; rows 0 / 64 stay zero
        # to swallow the denominator row of outT.
        pwP[p, 1 : 1 + HD, :] = pwT[a * HD : (a + 1) * HD, :]
        pwP[p, 65 : 65 + HD, :] = pwT[b * HD : (b + 1) * HD, :]

    # V blocks are [ones | v0..v47] per head so the softmax denominator lands
    # at a 32-aligned PSUM partition (0 / 64).
    wvA = np.zeros((C, VW), f32)
    vbA = np.zeros((1, VW), f32)
    for h in range(NH):
        wvA[:, h * (HD + 1) + 1 : (h + 1) * (HD + 1)] = vwT[:, h * HD : (h + 1) * HD]
        vbA[0, h * (HD + 1) + 1 : (h + 1) * (HD + 1)] = kv_b[
            C + h * HD : C + (h + 1) * HD
        ]
        vbA[0, h * (HD + 1)] = 1.0

    selE = np.zeros((1, 256), f32)
    selE[0, 0 : HD + 1] = 1.0
    selE[0, 128 + 64 : 128 + 64 + HD + 1] = 1.0

    mmdt = _mm_np_dtype()
    shared = {
        "selE": selE,
        "vbA": vbA.astype(mmdt),
        "qkbP": np.concatenate([qbP, kbP], axis=1),
        "pwP": pwP.astype(mmdt),
        "pbR": np.asarray(proj_b, f32).reshape(1, C).astype(mmdt),
    }
    # pack [x^T | wk | wq | wv] per core so the lead-in is 3 DMA triggers
    xkqv = np.concatenate(
        [xT, np.broadcast_to(np.stack([np.concatenate([wkP, wqP, wvA], axis=1)]), (B, C, XW - N))],
        axis=2,
    )
    return np.ascontiguousarray(xkqv).astype(mmdt), shared


_PROGRAM = None


def _get_program():
    global _PROGRAM
    if _PROGRAM is None:
        _PROGRAM = build_program(B)
    return _PROGRAM


def kernel(x, q_w, q_b, kv_w, kv_b, proj_w, proj_b):
    xT, shared = _prep_host(x, q_w, q_b, kv_w, kv_b, proj_w, proj_b)
    nc = _get_program()
    in_maps = [dict(shared, xT=np.ascontiguousarray(xT[b])) for b in range(B)]
    res = run_bass_kernel_spmd(nc, in_maps, list(range(B)))
    outs = [np.asarray(res.results[i]["out"], np.float32) for i in range(B)]
    return np.stack(outs).reshape(B, HH, WW, C)
